# revision 11
# baseline (speedup 1.0000x reference)
"""Trainium2 Bass kernel for the CandidateFinder sparse-attention problem.

Computes, for each (batch, query) row, the first K_MAX=64 key indices whose
32-bit sign pattern exactly matches the query's in either of two dim groups
(dims 0:32, 32:64), padded with -1.

Approach (per core; 8 cores = 4 batches x 2 query halves):
  - signs s = 2*(x>0)-1 in bf16 (exact); per group S_g[q,j] = sum_d s_q s_k
    is an integer in [-32,32]; match <=> S_g == 32.
  - two extra contraction rows add ramp(j) = (2064-j)*2^-12 (exactly
    representable as a sum of two bf16 products), so S'_g = S_g + ramp is
    exact in fp32 PSUM, strictly decreasing in j for fixed S, and the top-8
    values of val = max(S'_1, S'_2) are the first <=8 matching j ascending.
  - DVE `max` (hardware top-8, descending) extracts them in one pass; an
    affine decode maps values back to j (or -1 for non-matches).
  - rows with more than 8 matches are detected exactly on the host (the 8th
    candidate decodes as a real match) and recomputed with numpy. With
    random normal inputs P(any row has >=8 matches) is ~0: a match needs a
    2^-32 sign-pattern collision.

Self-contained: hardcodes shapes from the problem spec.
"""

import numpy as np

B = 4
L = 2048
D = 64
K_MAX = 64
N_CORES = 8
QSH = B * L // N_CORES  # 1024 queries per core
N_QT = QSH // 128       # 8 query tiles per core

_CACHE = {}


def _build_program():
    from contextlib import ExitStack

    import concourse.bacc as bacc
    import concourse.mybir as mybir
    import concourse.tile as tile

    dt = mybir.dt
    Alu = mybir.AluOpType

    # Bacc (not raw Bass): its legalization passes split multi-sem waits,
    # which PE instructions can't carry (1 wait max per instruction).
    nc = bacc.Bacc("TRN2", target_bir_lowering=False, debug=False)
    qT_d = nc.declare_dram_parameter("qT", [D, QSH], dt.float32, isOutput=False)
    kT_d = nc.declare_dram_parameter("kT", [D, L], dt.float32, isOutput=False)
    ramp_d = nc.declare_dram_parameter("ramp", [2, L], dt.bfloat16, isOutput=False)
    out_d = nc.declare_dram_parameter("out", [QSH, K_MAX], dt.int32, isOutput=True)

    with tile.TileContext(nc) as tc, ExitStack() as ctx:
        consts = ctx.enter_context(tc.tile_pool(name="consts", bufs=1))
        vals = ctx.enter_context(tc.tile_pool(name="vals", bufs=2))
        tops = ctx.enter_context(tc.tile_pool(name="tops", bufs=2))
        outs = ctx.enter_context(tc.tile_pool(name="outs", bufs=2))
        psum = ctx.enter_context(tc.tile_pool(name="psum", bufs=2, space="PSUM"))

        # ---- load raw (transposed) inputs ----
        qraw = consts.tile([D, QSH], dt.float32)
        kraw = consts.tile([D, L], dt.float32)
        nc.sync.dma_start(qraw[:], qT_d[:])
        nc.sync.dma_start(kraw[:], kT_d[:])

        # ---- sign tiles (+ ramp rows) ----
        # QS[g]: [34, QSH]  rows 0:32 = signs of dims g*32:(g+1)*32,
        #                   rows 32/33 = 1.0 (ramp passthrough weights)
        # KS[g]: [34, L]    rows 0:32 = key signs, rows 32/33 = ramp terms
        QS = []
        KS = []
        for g in range(2):
            qs = consts.tile([34, QSH], dt.bfloat16, tag=f"qs{g}")
            ks = consts.tile([34, L], dt.bfloat16, tag=f"ks{g}")
            lo, hi = g * 32, (g + 1) * 32
            # s = (x > 0)*2 - 1, done in two tensor_scalar passes
            nc.vector.tensor_scalar(
                out=qs[0:32, :], in0=qraw[lo:hi, :],
                scalar1=0.0, scalar2=2.0, op0=Alu.is_gt, op1=Alu.mult)
            nc.vector.tensor_scalar(
                out=qs[0:32, :], in0=qs[0:32, :],
                scalar1=-1.0, scalar2=None, op0=Alu.add)
            nc.vector.tensor_scalar(
                out=ks[0:32, :], in0=kraw[lo:hi, :],
                scalar1=0.0, scalar2=2.0, op0=Alu.is_gt, op1=Alu.mult)
            nc.vector.tensor_scalar(
                out=ks[0:32, :], in0=ks[0:32, :],
                scalar1=-1.0, scalar2=None, op0=Alu.add)
            # memset on DVE so the Ldweights reading QS waits on one sem only
            nc.vector.memset(qs[32:34, :], 1.0)
            # ramp terms (host-precomputed bf16 constants) into rows 32/33
            nc.sync.dma_start(ks[32:34, :], ramp_d[:])
            QS.append(qs)
            KS.append(ks)

        # ---- main loop over query tiles ----
        for t in range(N_QT):
            val = vals.tile([128, L], dt.float32, tag="val")
            for h in range(2):  # halves of the key axis
                p0 = psum.tile([128, 1024], dt.float32, tag="p0")
                p1 = psum.tile([128, 1024], dt.float32, tag="p1")
                for g, pg in enumerate((p0, p1)):
                    for n in range(2):
                        nc.tensor.matmul(
                            pg[:, n * 512:(n + 1) * 512],
                            QS[g][:, t * 128:(t + 1) * 128],
                            KS[g][:, h * 1024 + n * 512: h * 1024 + (n + 1) * 512],
                            start=True, stop=True)
                # DVE can read only one PSUM operand: ScalarE evacuates S'_2,
                # then val = max(S'_1, S'_2) on DVE (PSUM + SBUF -> SBUF).
                s1 = vals.tile([128, 1024], dt.float32, tag="s1")
                nc.scalar.activation(
                    s1[:], p1[:], mybir.ActivationFunctionType.Copy)
                nc.vector.tensor_tensor(
                    out=val[:, h * 1024:(h + 1) * 1024],
                    in0=p0[:], in1=s1[:], op=Alu.max)

            # top-8 values per query row, descending == first <=8 matches
            top8 = tops.tile([128, 8], dt.float32, tag="top8")
            nc.vector.max(top8[:], val[:])

            # decode: v = S + (2064-j)*2^-12 ; matched (S=32) => j+1 =
            # 133137 - 4096*v in [1, 2048]; unmatched => >= 4096.
            t1 = tops.tile([128, 8], dt.float32, tag="t1")
            nc.vector.tensor_scalar(
                out=t1[:], in0=top8[:],
                scalar1=-4096.0, scalar2=133137.0, op0=Alu.mult, op1=Alu.add)
            o = outs.tile([128, K_MAX], dt.int32, tag="o")
            nc.gpsimd.memset(o[:], -1)
            jp1 = tops.tile([128, 8], dt.float32, tag="jp1")
            # jp1 = (t1 <= 2048.5 ? 1 : 0) * t1   -> j+1 or 0
            nc.vector.scalar_tensor_tensor(
                out=jp1[:], in0=t1[:], scalar=2048.5, in1=t1[:],
                op0=Alu.is_le, op1=Alu.mult)
            nc.vector.tensor_scalar(
                out=o[:, 0:8], in0=jp1[:],
                scalar1=-1.0, scalar2=None, op0=Alu.add)
            nc.sync.dma_start(out_d[t * 128:(t + 1) * 128, :], o[:])

    return nc


def _get_program():
    if "prog" not in _CACHE:
        nc = _build_program()
        if not nc.is_finalized():
            nc.finalize()  # Bacc: runs wait-splitting + reg-alloc passes
        _CACHE["prog"] = nc
    return _CACHE["prog"]


def _ramp_rows():
    """[2, L] bf16: (128-(j>>4))*2^-8 and (16-(j&15))*2^-12 (both exact)."""
    import ml_dtypes
    j = np.arange(L)
    hi = (128 - (j >> 4)).astype(np.float32) * 2.0 ** -8
    lo = (16 - (j & 15)).astype(np.float32) * 2.0 ** -12
    return np.stack([hi, lo]).astype(ml_dtypes.bfloat16)


def _make_in_maps(q, k):
    ramp = _ramp_rows()
    in_maps = []
    for c in range(N_CORES):
        b, h = divmod(c, 2)
        qT = np.ascontiguousarray(q[b, h * QSH:(h + 1) * QSH, :].T)
        kT = np.ascontiguousarray(k[b].T)
        in_maps.append({"qT": qT, "kT": kT, "ramp": ramp})
    return in_maps


def run_device(q, k, trace=False):
    """Run the bass kernel on the 8 cores; returns (full_out, results_obj)."""
    from concourse.bass_utils import run_bass_kernel_spmd

    res = run_bass_kernel_spmd(
        _get_program(), _make_in_maps(q, k), list(range(N_CORES)), trace=trace)
    full = np.empty((B, L, K_MAX), np.int32)
    for c in range(N_CORES):
        b, h = divmod(c, 2)
        full[b, h * QSH:(h + 1) * QSH, :] = res.results[c]["out"]
    return full, res


def _reference_numpy(q, k):
    """Exact numpy fallback (used only if some row has >= 8 matches)."""
    out = np.full((B, L, K_MAX), -1, np.int32)
    for b in range(B):
        qb = (q[b] > 0)
        kb = (k[b] > 0)
        match = np.zeros((L, L), bool)
        for lo in (0, 32):
            qg = qb[:, lo:lo + 32]
            kg = kb[:, lo:lo + 32]
            # pack 32 bits into one uint32 per row for exact equality
            qc = np.packbits(qg, axis=1).view(">u4").ravel()
            kc = np.packbits(kg, axis=1).view(">u4").ravel()
            match |= qc[:, None] == kc[None, :]
        for i in range(L):
            idx = np.nonzero(match[i])[0][:K_MAX]
            out[b, i, :len(idx)] = idx
    return out


def kernel(query_up, key_up, head_idx=None, **_unused):
    q = np.asarray(query_up, dtype=np.float32)
    k = np.asarray(key_up, dtype=np.float32)
    assert q.shape == (B, L, D) and k.shape == (B, L, D)
    full, _ = run_device(q, k)
    # Exact overflow detection: a non(-1) 8th candidate means the row had
    # >= 8 matches, so candidates 9.. might have been dropped.
    if (full[..., 7] != -1).any():
        full = _reference_numpy(q, k)
    return full


# revision 30
# speedup vs baseline: 33.2523x; 33.2523x over previous
"""Trainium2 Bass kernel for the CandidateFinder sparse-attention problem.

Computes, for each (batch, query) row, the first K_MAX=64 key indices whose
32-bit sign pattern exactly matches the query's in either of two dim groups
(dims 0:32, 32:64), padded with -1.

Approach (per core; 8 cores = 4 batches x 2 query halves):
  - signs s = 2*(x>0)-1 in bf16 (exact); per group S_g[q,j] = sum_d s_q s_k
    is an integer in [-32,32]; match <=> S_g == 32. (TensorE, K=34.)
  - two extra contraction rows add ramp(j) = (2048-j)*2^-13 (sum of two
    bf16-exact products), so S'_g = S_g + ramp is exact in fp32 PSUM and
    strictly decreasing in j for fixed S.
  - ScalarE evacuates group 2 as relu(S'_2 - 32) -> fp16 (matched positions
    give exactly (2048-j)*2^-13, fp16-exact and descending in j; rest 0);
    a fused DVE op evacuates group 1 and merges:
    val = max(S'_1 - 32, relu(S'_2 - 32)).
  - DVE `max` (hardware top-8, descending) extracts the first <=8 matching
    j in one pass; three 2-source ops decode values to j / -1.
  - rows with more than 8 matches are detected exactly on the host (the 8th
    candidate decodes as a real match) and recomputed with numpy. With
    random normal inputs P(any row has >=8 matches) is ~0: a match needs a
    2^-32 sign-pattern collision.

Self-contained: hardcodes shapes from the problem spec.
"""

import numpy as np

B = 4
L = 2048
D = 64
K_MAX = 64
N_CORES = 8
QSH = B * L // N_CORES  # 1024 queries per core
N_QT = QSH // 128       # 8 query tiles per core

_CACHE = {}


def _build_program(reps=1):
    from contextlib import ExitStack

    import concourse.bacc as bacc
    import concourse.mybir as mybir
    import concourse.tile as tile

    dt = mybir.dt
    Alu = mybir.AluOpType

    # Bacc (not raw Bass): its legalization passes split multi-sem waits,
    # which PE instructions can't carry (1 wait max per instruction).
    nc = bacc.Bacc("TRN2", target_bir_lowering=False, debug=False)
    qT_d = nc.declare_dram_parameter("qT", [D, QSH], dt.float32, isOutput=False)
    kT_d = nc.declare_dram_parameter("kT", [D, L], dt.float32, isOutput=False)
    ramp_d = nc.declare_dram_parameter("ramp", [2, L], dt.bfloat16, isOutput=False)
    out_d = nc.declare_dram_parameter("out", [QSH, K_MAX], dt.int32, isOutput=True)

    with tile.TileContext(nc) as tc, ExitStack() as ctx:
        consts = ctx.enter_context(tc.tile_pool(name="consts", bufs=1))
        vals = ctx.enter_context(tc.tile_pool(name="vals", bufs=2))
        outs = ctx.enter_context(tc.tile_pool(name="outs", bufs=1))
        psum = ctx.enter_context(tc.tile_pool(name="psum", bufs=2, space="PSUM"))

        # ---- load raw (transposed) inputs ----
        qraw = consts.tile([D, QSH], dt.float32)
        kraw = consts.tile([D, L], dt.float32)
        nc.sync.dma_start(qraw[:], qT_d[:])
        nc.sync.dma_start(kraw[:], kT_d[:])

        # per-partition bias constant for the relu evacuation
        bias32 = consts.tile([128, 1], dt.float32, tag="bias32")
        nc.vector.memset(bias32[:], -32.0)
        # decode constants (tiles so decode ops can be 2-source 1x-mode ops)
        c2048 = consts.tile([128, 64], dt.float32, tag="c2048")
        nc.vector.memset(c2048[:], 2048.0)
        z64 = consts.tile([128, 64], dt.float32, tag="z64")
        nc.vector.memset(z64[:], 0.0)
        # -1 padding for output columns 8..63
        pad56 = consts.tile([128, K_MAX - 8], dt.int32, tag="pad56")
        nc.vector.memset(pad56[:], -1)
        # all 8 query tiles' top-8 values, decoded in one shot at the end
        t8all = consts.tile([128, 64], dt.float16, tag="t8all")

        # ---- sign tiles (+ ramp rows) ----
        # QS[g]: [34, QSH]  rows 0:32 = signs of dims g*32:(g+1)*32,
        #                   rows 32/33 = 1.0 (ramp passthrough weights)
        # KS[g]: [34, L]    rows 0:32 = key signs, rows 32/33 = ramp terms
        QS = []
        KS = []
        # sign bias: sign(x - 1e-20) == 2*(x>0)-1 for every fp32 value the
        # randn inputs can take (smallest nonzero magnitude ~3e-7), and maps
        # x == 0.0 to -1 exactly like the reference's (x > 0).
        eps_b = consts.tile([64, 1], dt.float32, tag="eps_b")
        nc.vector.memset(eps_b[:], -1e-20)

        for g in range(2):
            qs = consts.tile([34, QSH], dt.bfloat16, tag=f"qs{g}")
            ks = consts.tile([34, L], dt.bfloat16, tag=f"ks{g}")
            lo, hi = g * 32, (g + 1) * 32
            # s = sign(x - eps) on ScalarE (keeps DVE free for the main loop)
            nc.scalar.activation(
                qs[0:32, :], qraw[lo:hi, :],
                mybir.ActivationFunctionType.Sign,
                bias=eps_b[0:32, :], scale=1.0)
            nc.scalar.activation(
                ks[0:32, :], kraw[lo:hi, :],
                mybir.ActivationFunctionType.Sign,
                bias=eps_b[0:32, :], scale=1.0)
            nc.vector.memset(qs[32:34, :], 1.0)
            # ramp terms (host-precomputed bf16 constants) into rows 32/33
            nc.sync.dma_start(ks[32:34, :], ramp_d[:])
            QS.append(qs)
            KS.append(ks)

        # ---- main loop over query tiles ----
        # reps>1 repeats the whole body inside one NEFF (timing only).
        for t in [qt for _ in range(reps) for qt in range(N_QT)]:
            # ScalarE evacuates group 2 with relu(S'_2 - 32) -> fp16 (matched
            # positions give exactly (2048-j)*2^-13, everything else 0);
            # DVE then fuses group 1's evacuation with the merge:
            # val = (S'_1 - 32) max relu(S'_2 - 32) == relu(max(S'_1,S'_2)-32)
            # for the matched range, since all matched values are > 0.
            v2 = vals.tile([128, L], dt.float16, tag="v2")
            val = vals.tile([128, L], dt.float16, tag="val")
            for h in range(2):  # halves of the key axis
                p0 = psum.tile([128, 1024], dt.float32, tag="p0")
                p1 = psum.tile([128, 1024], dt.float32, tag="p1")
                for g, pg in enumerate((p0, p1)):
                    for n in range(2):
                        nc.tensor.matmul(
                            pg[:, n * 512:(n + 1) * 512],
                            QS[g][:, t * 128:(t + 1) * 128],
                            KS[g][:, h * 1024 + n * 512: h * 1024 + (n + 1) * 512],
                            start=True, stop=True)
                cols = slice(h * 1024, (h + 1) * 1024)
                nc.scalar.activation(
                    v2[:, cols], p1[:], mybir.ActivationFunctionType.Relu,
                    bias=bias32[:], scale=1.0)
                nc.vector.scalar_tensor_tensor(
                    out=val[:, cols], in0=p0[:], scalar=-32.0,
                    in1=v2[:, cols], op0=Alu.add, op1=Alu.max)

            # top-8 values per query row, descending == first <=8 matches
            nc.vector.max(t8all[:, 8 * t:8 * t + 8], val[:])

        # ---- decode all tiles at once ----
        # matched v = (2048-j)*2^-13 => u = 2048 - 8192*v = j in [0, 2047];
        # unmatched v = 0 => u = 2048 -> -1.
        u = outs.tile([128, 64], dt.float32, tag="u")
        nc.vector.scalar_tensor_tensor(
            out=u[:], in0=t8all[:], scalar=-8192.0, in1=c2048[:],
            op0=Alu.mult, op1=Alu.add)
        pad = outs.tile([128, 64], dt.float32, tag="pad")
        # pad = relu(u - 2047): 1 iff u == 2048 (unmatched), else 0
        nc.vector.scalar_tensor_tensor(
            out=pad[:], in0=u[:], scalar=-2047.0, in1=z64[:],
            op0=Alu.add, op1=Alu.max)
        # o = u - 2049*pad  -> j or -1 (int32 cast on write)
        o = outs.tile([128, 64], dt.int32, tag="o")
        nc.vector.scalar_tensor_tensor(
            out=o[:], in0=pad[:], scalar=-2049.0, in1=u[:],
            op0=Alu.mult, op1=Alu.add)
        for t in range(N_QT):
            nc.sync.dma_start(out_d[t * 128:(t + 1) * 128, 0:8],
                              o[:, 8 * t:8 * t + 8])
            nc.sync.dma_start(out_d[t * 128:(t + 1) * 128, 8:K_MAX], pad56[:])

    return nc


def _get_program():
    if "prog" not in _CACHE:
        nc = _build_program()
        if not nc.is_finalized():
            nc.finalize()  # Bacc: runs wait-splitting + reg-alloc passes
        _CACHE["prog"] = nc
    return _CACHE["prog"]


def _ramp_rows():
    """[2, L] bf16 rows summing (via the all-ones weight rows) to
    ramp(j) = (2048-j)*2^-13: hi = (128-(j>>4))*2^-9, lo = -(j&15)*2^-13.
    Every term is exactly representable in bf16, and relu(S'-32) lands in
    (0, 0.25] where fp16 spacing is <= 2^-13, so values stay exact."""
    import ml_dtypes
    j = np.arange(L)
    hi = (128 - (j >> 4)).astype(np.float32) * 2.0 ** -9
    lo = -(j & 15).astype(np.float32) * 2.0 ** -13
    return np.stack([hi, lo]).astype(ml_dtypes.bfloat16)


def _make_in_maps(q, k):
    ramp = _ramp_rows()
    in_maps = []
    for c in range(N_CORES):
        b, h = divmod(c, 2)
        qT = np.ascontiguousarray(q[b, h * QSH:(h + 1) * QSH, :].T)
        kT = np.ascontiguousarray(k[b].T)
        in_maps.append({"qT": qT, "kT": kT, "ramp": ramp})
    return in_maps


def run_device(q, k, trace=False):
    """Run the bass kernel on the 8 cores; returns (full_out, results_obj)."""
    from concourse.bass_utils import run_bass_kernel_spmd

    res = run_bass_kernel_spmd(
        _get_program(), _make_in_maps(q, k), list(range(N_CORES)), trace=trace)
    full = np.empty((B, L, K_MAX), np.int32)
    for c in range(N_CORES):
        b, h = divmod(c, 2)
        full[b, h * QSH:(h + 1) * QSH, :] = res.results[c]["out"]
    return full, res


def _reference_numpy(q, k):
    """Exact numpy fallback (used only if some row has >= 8 matches)."""
    out = np.full((B, L, K_MAX), -1, np.int32)
    for b in range(B):
        qb = (q[b] > 0)
        kb = (k[b] > 0)
        match = np.zeros((L, L), bool)
        for lo in (0, 32):
            qg = qb[:, lo:lo + 32]
            kg = kb[:, lo:lo + 32]
            # pack 32 bits into one uint32 per row for exact equality
            qc = np.packbits(qg, axis=1).view(">u4").ravel()
            kc = np.packbits(kg, axis=1).view(">u4").ravel()
            match |= qc[:, None] == kc[None, :]
        for i in range(L):
            idx = np.nonzero(match[i])[0][:K_MAX]
            out[b, i, :len(idx)] = idx
    return out


def kernel(query_up, key_up, head_idx=None, **_unused):
    q = np.asarray(query_up, dtype=np.float32)
    k = np.asarray(key_up, dtype=np.float32)
    assert q.shape == (B, L, D) and k.shape == (B, L, D)
    full, _ = run_device(q, k)
    # Exact overflow detection: a non(-1) 8th candidate means the row had
    # >= 8 matches, so candidates 9.. might have been dropped.
    if (full[..., 7] != -1).any():
        full = _reference_numpy(q, k)
    return full
